# revision 24
# baseline (speedup 1.0000x reference)
"""MultiHeadAttention Trainium2 Bass kernel (8-core SPMD), v2 pipelined.

Problem: B=2, S=2048, DIM=1024, H=16 heads (dh=64), fp32 reference.
Sharding: core c handles batch b = c//4 and 4 heads ho = 4*(c%4)..+4.

v2 structure (vs v1's serial proj -> attention -> norm -> outproj):
everything is software-pipelined under the ScalarE exp stream, which is
the hard floor (128 exps x [128,1024] ~= 147us).  q is processed in
256-column chunks (NQC=8) so PSUM fits: scores 4 banks + PV 2 banks +
aux (proj/outproj) 2 banks = 8.  Per q-chunk: scores (row-group-packed
K=64 pairs) -> exp (ACT) -> keep-mask mult (DVE, fused over kt pairs) ->
PV with fused ones-column row-sums -> per-chunk normalization via
SBUF gather -> reciprocal -> DRAM broadcast (overlapped, no stall) ->
out-projection + output DMA.  Projections for k/v/q trail the input
DMAs c-wave by c-wave during the ramp; leftover v/q projections and
all norm/outproj work are emitted into the engine FIFOs with explicit
lag so nothing head-of-line blocks the exp chain.

x/w/mask tiles are host-packed so DMAs move 2-4KB per partition row
(the 1KB-row DMAs of v1 measured only ~200GB/s aggregate).
"""

import os
import sys

sys.path.insert(0, "/opt/trn_rl_repo")
os.environ.setdefault("MYCRO_LOCAL_CACHE", "1")

import numpy as np

import concourse.bass as bass
import concourse.bacc as bacc
import concourse.tile as tile
from concourse import mybir
from concourse import bass_utils

F32 = mybir.dt.float32
BF16 = mybir.dt.bfloat16
NP_BF16 = mybir.dt.np(BF16)

B, S, DIM = 2, 2048, 1024
H = 16
DH = 64
SCALE = 1.0 / (DIM ** 0.5)
N_CORES = 8
HPC = 4            # heads per core
QC = 256           # q-chunk
NQC = S // QC      # 8 q-chunks
KT = S // 128      # 16 k-tiles of 128
CT = DIM // 128    # 8 contraction tiles for projections
VHA_W = 386        # [vhA|1 | 0x32 1 0x31 vhB] x 2 pairs (65+128 per pair)


def _bc(ap_obj, dims):
    """Rebuild an AP with explicit dims (list of [step, num])."""
    return bass.AP(tensor=ap_obj.tensor, offset=ap_obj.offset, ap=dims)


def build_nc():
    nc = bacc.Bacc("TRN2", target_bir_lowering=False)

    xk_d = nc.declare_dram_parameter("xk", [CT, 128, S], BF16, isOutput=False)
    xv_d = nc.declare_dram_parameter("xv", [CT, 128, S], BF16, isOutput=False)
    xq_d = nc.declare_dram_parameter("xq", [CT, 128, S], BF16, isOutput=False)
    wq_d = nc.declare_dram_parameter("wq", [128, 2048], BF16, isOutput=False)
    wk_d = nc.declare_dram_parameter("wk", [128, 2048], BF16, isOutput=False)
    wv_d = nc.declare_dram_parameter("wv", [128, 2048], BF16, isOutput=False)
    wo_d = nc.declare_dram_parameter("wo", [128, 2048], BF16, isOutput=False)
    bq_d = nc.declare_dram_parameter("bq2", [2, 128, 1], F32, isOutput=False)
    bk_d = nc.declare_dram_parameter("bk2", [2, 128, 1], F32, isOutput=False)
    bvb_d = nc.declare_dram_parameter("bvb", [128, 256], BF16, isOutput=False)
    mk_d = nc.declare_dram_parameter("mk", [NQC, 4, 128, 1024], BF16,
                                     isOutput=False)
    yt_d = nc.declare_dram_parameter("yt", [NQC, 128, 2048], BF16,
                                     isOutput=True)
    rscr_d = nc.dram_tensor("rscr", [NQC, 1024], F32)

    with tile.TileContext(nc) as tc:
        with tc.tile_pool(name="sb", bufs=1) as sb, \
             tc.tile_pool(name="ps", bufs=2, space="PSUM") as ps:

            # ---------------- DMA: biases + weights first ----------------
            bq_sb, bk_sb = [], []
            for m in range(2):
                t = sb.tile([128, 1], F32, tag=f"bq{m}", name=f"bq{m}")
                nc.sync.dma_start(out=t, in_=bq_d[m])
                bq_sb.append(t)
                t = sb.tile([128, 1], F32, tag=f"bk{m}", name=f"bk{m}")
                nc.sync.dma_start(out=t, in_=bk_d[m])
                bk_sb.append(t)
            bvb_sb = sb.tile([128, 256], BF16, tag="bvb")
            nc.sync.dma_start(out=bvb_sb, in_=bvb_d[:, :])

            # DMA issue order = arrival order = the order compute needs it:
            # wk -> xk (kproj) -> wv -> xv (vproj) -> masks 0-2 -> wq ->
            # xq-n0 (qproj/scores) -> masks 3-5 -> wo.
            wk_sb = sb.tile([128, 2048], BF16, tag="wk")
            nc.sync.dma_start(out=wk_sb, in_=wk_d[:, :])
            xk_sb = []
            for c in range(CT):
                t = sb.tile([128, S], BF16, tag=f"xk{c}", name=f"xk{c}")
                nc.sync.dma_start(out=t, in_=xk_d[c])
                xk_sb.append(t)
            wv_sb = sb.tile([128, 2048], BF16, tag="wv")
            nc.sync.dma_start(out=wv_sb, in_=wv_d[:, :])
            xv_sb = []
            for c in range(CT):
                t = sb.tile([128, S], BF16, tag=f"xv{c}", name=f"xv{c}")
                nc.sync.dma_start(out=t, in_=xv_d[c])
                xv_sb.append(t)

            # masks: tile index M = qc*4 + g covers kt 4g..4g+3 of chunk qc.
            mk_sb = {}

            def mask_dma(M):
                qc, g = M // 4, M % 4
                t = sb.tile([128, 1024], BF16, tag="mask", name="mask", bufs=6)
                nc.sync.dma_start(out=t, in_=mk_d[qc, g])
                mk_sb[M] = t

            for M in range(3):
                mask_dma(M)

            wq_sb = sb.tile([128, 2048], BF16, tag="wq")
            nc.sync.dma_start(out=wq_sb, in_=wq_d[:, :])
            xq_sb = []
            for c in range(CT):
                t = sb.tile([128, S], BF16, tag=f"xq{c}", name=f"xq{c}")
                nc.sync.dma_start(out=t[:, 0:512], in_=xq_d[c][:, 0:512])
                xq_sb.append(t)
            for M in range(3, 6):
                mask_dma(M)
            wo_sb = sb.tile([128, 2048], BF16, tag="wo")
            nc.sync.dma_start(out=wo_sb, in_=wo_d[:, :])

            # ---------------- persistent SBUF intermediates ----------------
            khT = [[sb.tile([128, 512], BF16, tag=f"khT{m}_{n}",
                            name=f"khT{m}_{n}") for n in range(4)]
                   for m in range(2)]
            qhT = [[sb.tile([128, 512], BF16, tag=f"qhT{m}_{n}",
                            name=f"qhT{m}_{n}") for n in range(4)]
                   for m in range(2)]
            vha = [sb.tile([128, VHA_W], BF16, tag=f"vha{kt}",
                           name=f"vha{kt}") for kt in range(KT)]
            OT = [sb.tile([128, S], BF16, tag=f"OT{p}", name=f"OT{p}")
                  for p in range(2)]

            # ---------------- PE warmup + exp table prewarm (FIRST: the
            # warm memset gates the warmup matmuls, so it must precede the
            # 64 vha memsets on the gpsimd queue) ----------------
            warm = sb.tile([128, 512], BF16, tag="warm")
            nc.gpsimd.memset(warm[:, :], 0.0)
            wps = ps.tile([128, 512], F32, tag="ax", name="wps")
            for i in range(12):
                nc.tensor.matmul(wps, warm[:, 0:128], warm[:, :],
                                 start=True, stop=True)
            prew = sb.tile([128, 32], BF16, tag="prew")
            nc.scalar.activation(out=prew, in_=warm[:, 0:32],
                                 func=mybir.ActivationFunctionType.Exp,
                                 scale=1.0)

            for kt in range(KT):
                for p in range(2):
                    base = p * 193
                    nc.gpsimd.memset(vha[kt][:, base + 64:base + 65], 1.0)
                    nc.gpsimd.memset(vha[kt][:, base + 97:base + 98], 1.0)
                    nc.gpsimd.memset(vha[kt][:, base + 65:base + 97], 0.0)
                    nc.gpsimd.memset(vha[kt][:, base + 98:base + 129], 0.0)

            # ---------------- k projection: 8 groups, c-outer ----------------
            # groups (m, n): n0 -> sc-tile halves, n1 -> sc, n2 -> po, n3 -> ax
            psk = [ps.tile([128, 1024], F32, tag="sc", name=f"psk{n}")
                   for n in range(2)]
            pso = [ps.tile([128, 512], F32, tag="po", name=f"pso{m}")
                   for m in range(2)]
            psa = [ps.tile([128, 512], F32, tag="ax", name=f"psa{m}")
                   for m in range(2)]

            def kgroup_ap(m, n):
                if n < 2:
                    return psk[n][:, m * 512:(m + 1) * 512]
                return (pso if n == 2 else psa)[m][:, :]

            for c in range(CT):
                for n in range(4):
                    for m in range(2):
                        nc.tensor.matmul(
                            kgroup_ap(m, n),
                            wk_sb[:, c * 256 + m * 128:c * 256 + (m + 1) * 128],
                            xk_sb[c][:, n * 512:(n + 1) * 512],
                            start=(c == 0), stop=(c == CT - 1))
            for n in range(4):
                for m in range(2):
                    bb = bk_sb[m][:, 0:1]
                    nc.vector.tensor_tensor(
                        out=khT[m][n], in0=kgroup_ap(m, n),
                        in1=_bc(bb, [list(bb.ap[0]), [0, 512]]),
                        op=mybir.AluOpType.add)

            # ---------------- v projection: c-outer waves over kt ----------
            # 8 kt-pair psum groups spread over all three tags (ramp only):
            # j0..3 -> two sc tiles (halves), j4,j5 -> po, j6,j7 -> ax.
            vsc = [ps.tile([128, 1024], F32, tag="sc", name=f"vsc{i}")
                   for i in range(2)]
            vpo = [ps.tile([128, 512], F32, tag="po", name=f"vpo{i}")
                   for i in range(2)]
            vax = [ps.tile([128, 512], F32, tag="ax", name=f"vax{i}")
                   for i in range(2)]

            def vps(j):
                if j < 4:
                    return vsc[j // 2][:, (j % 2) * 512:(j % 2 + 1) * 512]
                if j < 6:
                    return vpo[j - 4][:, :]
                return vax[j - 6][:, :]

            for c in range(CT):
                for j in range(KT // 2):
                    for kk in range(2):
                        # one start/stop per BANK: start only on (c0, kk0),
                        # stop only on (c7, kk1); flags=0 overwrites regions
                        # whose has_written bit is clear, so kk1@c0 is safe.
                        kt = 2 * j + kk
                        nc.tensor.matmul(
                            vps(j)[:, kk * 256:(kk + 1) * 256],
                            xv_sb[c][:, kt * 128:(kt + 1) * 128],
                            wv_sb[:, c * 256:(c + 1) * 256],
                            start=(c == 0 and kk == 0),
                            stop=(c == CT - 1 and kk == 1))
            def vha_cast(j, kk):
                    kt = 2 * j + kk
                    # A blocks (heads 0,2) then B blocks (heads 1,3), 3D APs
                    src = vps(j)[:, kk * 256:kk * 256 + 64]
                    dstA = vha[kt][:, 0:64]
                    nc.vector.tensor_tensor(
                        out=_bc(dstA, [list(dstA.ap[0]), [193, 2], [1, 64]]),
                        in0=_bc(src, [list(src.ap[0]), [128, 2], [1, 64]]),
                        in1=_bc(bvb_sb[:, 0:64],
                                [list(bvb_sb.ap[0]), [128, 2], [1, 64]]),
                        op=mybir.AluOpType.add)
                    srcB = vps(j)[:, kk * 256 + 64:kk * 256 + 128]
                    dstB = vha[kt][:, 129:193]
                    nc.vector.tensor_tensor(
                        out=_bc(dstB, [list(dstB.ap[0]), [193, 2], [1, 64]]),
                        in0=_bc(srcB, [list(srcB.ap[0]), [128, 2], [1, 64]]),
                        in1=_bc(bvb_sb[:, 64:128],
                                [list(bvb_sb.ap[0]), [128, 2], [1, 64]]),
                        op=mybir.AluOpType.add)
            # NOTE: vha_cast emission deferred until after the qproj-n0
            # casts so the 32 vha casts don't delay qhT on the DVE FIFO.

            # ---------------- q projection chunk n ----------------
            qpq = {}

            def qproj_step(n, c):
                # one c-wave (2 MMs); c==0 allocates, c==CT-1 adds the casts
                if c == 0:
                    qpq[n] = [ps.tile([128, 512], F32, tag="ax",
                                      name=f"pq{n}_{m}") for m in range(2)]
                for m in range(2):
                    nc.tensor.matmul(
                        qpq[n][m],
                        wq_sb[:, c * 256 + m * 128:c * 256 + (m + 1) * 128],
                        xq_sb[c][:, n * 512:(n + 1) * 512],
                        start=(c == 0), stop=(c == CT - 1))
                if c == CT - 1:
                    for m in range(2):
                        bb = bq_sb[m][:, 0:1]
                        nc.vector.tensor_tensor(
                            out=qhT[m][n], in0=qpq[n][m],
                            in1=_bc(bb, [list(bb.ap[0]), [0, 512]]),
                            op=mybir.AluOpType.add)

            for c in range(CT):
                qproj_step(0, c)
            # vha casts AFTER the qproj casts (DVE FIFO order): scores only
            # need khT/qhT; vha is first needed by PV at slot 6 of chunk 0.
            for j in range(KT // 2):
                for kk in range(2):
                    vha_cast(j, kk)

            # ---------------- attention + trailing work ----------------
            # sc col layout per (qc, kt): [h0 | h2 | h1 | h3] x 256 q.
            A_COL = {0: 0, 1: 256}        # pair -> A-head col base
            B_COL = {0: 512, 1: 768}      # pair -> B-head col base

            po = {}        # (qc, p) -> psum tile
            pt2 = {}       # (qc, ktpair) -> sbuf tile
            rbc = {}       # (qc, p) -> sbuf [128, 256] f32
            state = {}

            def emit_scores(qc, kt):
                sc = ps.tile([128, 1024], F32, tag="sc", name="sc")
                n, qoff = qc // 2, (qc % 2) * 256
                kslice = slice((kt % 4) * 128, (kt % 4 + 1) * 128)
                for p in range(2):
                    for ab in range(2):
                        rows = slice(ab * 64, (ab + 1) * 64)
                        col = A_COL[p] if ab == 0 else B_COL[p]
                        nc.tensor.matmul(
                            sc[:, col:col + 256],
                            khT[p][kt // 4][rows, kslice],
                            qhT[p][n][rows, qoff:qoff + 256],
                            start=True, stop=True)
                return sc

            def emit_exp_mask(qc, kt, sc):
                j = kt // 2
                if kt % 2 == 0:
                    pt2[(qc, j)] = sb.tile([128, 2048], BF16, tag="pt2",
                                           name="pt2", bufs=5)
                t = pt2[(qc, j)]
                nc.scalar.activation(
                    out=t[:, (kt % 2) * 1024:(kt % 2 + 1) * 1024], in_=sc,
                    func=mybir.ActivationFunctionType.Exp, scale=float(SCALE))
                if kt % 2 == 1:
                    mkt = mk_sb[qc * 4 + kt // 4]
                    moff = ((kt - 1) % 4) * 256
                    msrc = mkt[:, moff:moff + 256]
                    nc.vector.tensor_tensor(
                        out=t, in0=t,
                        in1=_bc(msrc, [list(msrc.ap[0]), [256, 2], [0, 4],
                                       [1, 256]]),
                        op=mybir.AluOpType.mult)

            def emit_pv(qc, kt):
                t = pt2[(qc, kt // 2)]
                base = (kt % 2) * 1024
                for p in range(2):
                    # A and B share one bank: single start (A@kt0) / stop
                    # (B@kt15); B@kt0 overwrites via clear has_written bits.
                    # A uses M=128 (into the B-block zeros) so the start
                    # marks every partition of the bank; partitions 65:128
                    # of cols 0:256 accumulate unused garbage.
                    vb = p * 193
                    nc.tensor.matmul(
                        po[(qc, p)][:, 0:256],
                        vha[kt][:, vb:vb + 128],
                        t[:, base + A_COL[p]:base + A_COL[p] + 256],
                        start=(kt == 0), stop=False)
                    nc.tensor.matmul(
                        po[(qc, p)][:, 256:512],
                        vha[kt][:, vb + 65:vb + 193],
                        t[:, base + B_COL[p]:base + B_COL[p] + 256],
                        start=False, stop=(kt == KT - 1))

            def emit_sums(qc):
                t = sb.tile([1, 1024], F32, tag="ss", name="ss", bufs=2)
                for p in range(2):
                    nc.vector.tensor_copy(
                        out=t[0:1, p * 512:p * 512 + 256],
                        in_=po[(qc, p)][64:65, 0:256])
                    nc.vector.tensor_copy(
                        out=t[0:1, p * 512 + 256:(p + 1) * 512],
                        in_=po[(qc, p)][32:33, 256:512])
                state[("ss", qc)] = t
                # norm-chain DMAs on the sync HWDGE queue (SWDGE measured
                # ~5us extra latency per hop); emitted early (kt 0/2) so
                # their queue-head waits are short.
                rg = sb.tile([128, 8], F32, tag="rg", name="rg", bufs=2)
                nc.sync.dma_start(out=rg, in_=t[0:1, :])
                state[("rg", qc)] = rg

            def emit_recip(qc):
                rr = sb.tile([128, 8], F32, tag="rr", name="rr", bufs=2)
                nc.vector.reciprocal(out=rr, in_=state[("rg", qc)])
                nc.sync.dma_start(out=rscr_d[qc], in_=rr)
                for p in range(2):
                    t = sb.tile([128, 256], F32, tag="rbc", name="rbc", bufs=4)
                    # one DMA per pair: 3D src (hb, 64-part block, q)
                    srow = rscr_d[qc:qc + 1, p * 512:(p + 1) * 512]
                    nc.sync.dma_start(
                        out=t,
                        in_=_bc(srow, [[256, 2], [0, 64], [1, 256]]))
                    rbc[(qc, p)] = t

            def emit_ot(qc, p):
                qsl = slice(qc * 256, (qc + 1) * 256)
                nc.vector.tensor_tensor(
                    out=OT[p][0:64, qsl],
                    in0=po[(qc, p)][0:64, 0:256],
                    in1=rbc[(qc, p)][0:64, :],
                    op=mybir.AluOpType.mult)
                nc.vector.tensor_tensor(
                    out=OT[p][64:128, qsl],
                    in0=po[(qc, p)][64:128, 256:512],
                    in1=rbc[(qc, p)][64:128, :],
                    op=mybir.AluOpType.mult)

            yts_cur = {}

            def emit_outproj(qc, otp):
                ax = ps.tile([128, 512], F32, tag="ax", name="axo")
                for half in range(2):
                    ot = 2 * otp + half
                    for p in range(2):
                        nc.tensor.matmul(
                            ax[:, half * 256:(half + 1) * 256],
                            wo_sb[:, p * 1024 + ot * 128:
                                  p * 1024 + (ot + 1) * 128],
                            OT[p][:, qc * 256:(qc + 1) * 256],
                            start=(p == 0), stop=(p == 1))
                if otp == 0:
                    yts_cur[qc] = sb.tile([128, 2048], BF16, tag="yts",
                                          name="yts", bufs=2)
                yts = yts_cur[qc]
                nc.vector.tensor_copy(
                    out=yts[:, otp * 512:(otp + 1) * 512], in_=ax)
                if otp == 3:
                    nc.sync.dma_start(out=yt_d[qc], in_=yts)

            # main loop: per chunk qc, slots kt=0..15 pace the emission.
            for qc in range(NQC):
                for p in range(2):
                    po[(qc, p)] = ps.tile([128, 512], F32, tag="po",
                                          name=f"po{p}")
                for kt in range(KT):
                    u = qc * KT + kt  # global slot
                    # JIT DMAs (at u%4==3 so they queue behind, not ahead
                    # of, the latency-critical norm-chain DMAs at kt 0/2)
                    if u % 4 == 3:
                        M = u // 4 + 6
                        if M < NQC * 4:
                            mask_dma(M)
                    if u == 3:
                        for c in range(CT):
                            nc.sync.dma_start(out=xq_sb[c][:, 512:2048],
                                              in_=xq_d[c][:, 512:2048])
                    sc = emit_scores(qc, kt)
                    emit_exp_mask(qc, kt, sc)
                    # deferred norm/outproj for previous chunk; emitted as
                    # early as its dependencies allow so the po banks free
                    # before PV(qc, 0) at slot 6.
                    if qc > 0:
                        pq = qc - 1
                        if kt == 0:
                            emit_sums(pq)
                        elif kt == 2:
                            emit_recip(pq)
                        elif kt == 4:
                            emit_ot(pq, 0)
                        elif kt == 5:
                            emit_ot(pq, 1)
                        elif kt in (7, 9, 11, 13):
                            emit_outproj(pq, (kt - 7) // 2)
                    if qc in (1, 3, 5) and kt < CT:
                        qproj_step((qc + 1) // 2, kt)
                    # PV with lag 6 (waits po release by norm of qc-1)
                    if kt >= 6:
                        emit_pv(qc, kt - 6)
                for kt in range(KT - 6, KT):
                    emit_pv(qc, kt)
            emit_sums(NQC - 1)
            emit_recip(NQC - 1)
            emit_ot(NQC - 1, 0)
            emit_ot(NQC - 1, 1)
            for otp in range(4):
                emit_outproj(NQC - 1, otp)

    nc.compile()
    return nc


_NC_CACHE = None


def get_nc():
    global _NC_CACHE
    if _NC_CACHE is None:
        _NC_CACHE = build_nc()
    return _NC_CACHE


def prep_in_maps(q, k, v, mask, Wq, bq, Wk, bk, Wv, bv, Wo, bo):
    q = np.asarray(q, np.float32)
    k = np.asarray(k, np.float32)
    v = np.asarray(v, np.float32)
    mask = np.asarray(mask)
    WqT = np.asarray(Wq, np.float32).T
    WkT = np.asarray(Wk, np.float32).T
    WvT = np.asarray(Wv, np.float32).T
    WoT = np.asarray(Wo, np.float32).T
    bq = np.asarray(bq, np.float32)
    bk = np.asarray(bk, np.float32)
    bv = np.asarray(bv, np.float32)

    xT = {}
    mkw = {}
    for b in range(B):
        xT[b] = tuple(
            np.ascontiguousarray(a.T).astype(NP_BF16).reshape(CT, 128, S)
            for a in (k[b], v[b], q[b]))
        keep = np.ascontiguousarray(
            (~mask[b, 0]).T.astype(np.float32)).astype(NP_BF16)  # [kpos, q]
        # [g, j, p, qc, q256] -> [qc, g, p, j*256]
        a = keep.reshape(4, 4, 128, NQC, 256)
        mkw[b] = np.ascontiguousarray(
            a.transpose(3, 0, 2, 1, 4).reshape(NQC, 4, 128, 1024))

    def wpack(WT, dsl):
        # [1024, 256] -> [128, 2048] with cols c*256+j
        return np.ascontiguousarray(
            WT[:, dsl].reshape(CT, 128, 256).transpose(1, 0, 2)
            .reshape(128, 2048)).astype(NP_BF16)

    in_maps = []
    for c in range(N_CORES):
        b = c // 4
        ho = c % 4
        dsl = slice(ho * 256, ho * 256 + 256)
        xk, xv, xq = xT[b]
        in_maps.append({
            "xk": xk, "xv": xv, "xq": xq,
            "wq": wpack(WqT, dsl),
            "wk": wpack(WkT, dsl),
            "wv": wpack(WvT, dsl),
            "wo": np.ascontiguousarray(
                WoT[dsl, :].reshape(2, 128, 1024).transpose(1, 0, 2)
                .reshape(128, 2048)).astype(NP_BF16),
            "bq2": np.ascontiguousarray(bq[dsl]).reshape(2, 128, 1)
                .astype(np.float32),
            "bk2": np.ascontiguousarray(bk[dsl]).reshape(2, 128, 1)
                .astype(np.float32),
            "bvb": np.ascontiguousarray(
                np.broadcast_to(bv[dsl], (128, 256))).astype(NP_BF16),
            "mk": mkw[b],
        })
    return in_maps


def assemble_yT(yt):
    # yt [NQC, 128, 2048] -> yT [1024, 2048]; cols = otp*512 + half*256 + q,
    # y-dim = otp*256 + half*128 + part
    a = np.asarray(yt, np.float32).reshape(NQC, 128, 4, 2, 256)
    return a.transpose(2, 3, 1, 0, 4).reshape(DIM, S)


def gather_output(results, bo):
    bo = np.asarray(bo, np.float32)
    y = np.zeros((B, S, DIM), np.float32)
    for c in range(N_CORES):
        y[c // 4] += assemble_yT(results[c]["yt"]).T
    y += bo[None, None, :]
    return y


def kernel(**inputs):
    nc = get_nc()
    in_maps = prep_in_maps(**{k_: inputs[k_] for k_ in (
        "q", "k", "v", "mask", "Wq", "bq", "Wk", "bk", "Wv", "bv", "Wo", "bo")})
    res = bass_utils.run_bass_kernel_spmd(nc, in_maps, list(range(N_CORES)))
    return gather_output(res.results, inputs["bo"])


# revision 31
# speedup vs baseline: 1.1367x; 1.1367x over previous
"""MultiHeadAttention Trainium2 Bass kernel (8-core SPMD), v2 pipelined.

Problem: B=2, S=2048, DIM=1024, H=16 heads (dh=64), fp32 reference.
Sharding: core c handles batch b = c//4 and 4 heads ho = 4*(c%4)..+4.

v2 structure (vs v1's serial proj -> attention -> norm -> outproj):
everything is software-pipelined under the ScalarE exp stream, which is
the hard floor (128 exps x [128,1024] ~= 147us).  q is processed in
256-column chunks (NQC=8) so PSUM fits: scores 4 banks + PV 2 banks +
aux (proj/outproj) 2 banks = 8.  Per q-chunk: scores (row-group-packed
K=64 pairs) -> exp (ACT) -> keep-mask mult (DVE, fused over kt pairs) ->
PV with fused ones-column row-sums -> per-chunk normalization via
SBUF gather -> reciprocal -> DRAM broadcast (overlapped, no stall) ->
out-projection + output DMA.  Projections for k/v/q trail the input
DMAs c-wave by c-wave during the ramp; leftover v/q projections and
all norm/outproj work are emitted into the engine FIFOs with explicit
lag so nothing head-of-line blocks the exp chain.

x/w/mask tiles are host-packed so DMAs move 2-4KB per partition row
(the 1KB-row DMAs of v1 measured only ~200GB/s aggregate).
"""

import os
import sys

sys.path.insert(0, "/opt/trn_rl_repo")
os.environ.setdefault("MYCRO_LOCAL_CACHE", "1")

import numpy as np

import concourse.bass as bass
import concourse.bacc as bacc
import concourse.tile as tile
from concourse import mybir
from concourse import bass_utils

F32 = mybir.dt.float32
BF16 = mybir.dt.bfloat16
NP_BF16 = mybir.dt.np(BF16)

B, S, DIM = 2, 2048, 1024
H = 16
DH = 64
SCALE = 1.0 / (DIM ** 0.5)
N_CORES = 8
HPC = 4            # heads per core
QC = 256           # q-chunk
NQC = S // QC      # 8 q-chunks
KT = S // 128      # 16 k-tiles of 128
CT = DIM // 128    # 8 contraction tiles for projections
VHA_W = 386        # [vhA|1 | 0x32 1 0x31 vhB] x 2 pairs (65+128 per pair)


def _bc(ap_obj, dims):
    """Rebuild an AP with explicit dims (list of [step, num])."""
    return bass.AP(tensor=ap_obj.tensor, offset=ap_obj.offset, ap=dims)


def build_nc():
    nc = bacc.Bacc("TRN2", target_bir_lowering=False)

    xk_d = nc.declare_dram_parameter("xk", [CT, 128, S], BF16, isOutput=False)
    xv_d = nc.declare_dram_parameter("xv", [CT, 128, S], BF16, isOutput=False)
    xq_d = nc.declare_dram_parameter("xq", [CT, 128, S], BF16, isOutput=False)
    wq_d = nc.declare_dram_parameter("wq", [128, 2048], BF16, isOutput=False)
    wk_d = nc.declare_dram_parameter("wk", [128, 2048], BF16, isOutput=False)
    wv_d = nc.declare_dram_parameter("wv", [128, 2048], BF16, isOutput=False)
    wo_d = nc.declare_dram_parameter("wo", [128, 2048], BF16, isOutput=False)
    bq_d = nc.declare_dram_parameter("bq2", [2, 128, 1], F32, isOutput=False)
    bk_d = nc.declare_dram_parameter("bk2", [2, 128, 1], F32, isOutput=False)
    bvb_d = nc.declare_dram_parameter("bvb", [128, 256], BF16, isOutput=False)
    mk_d = nc.declare_dram_parameter("mk", [NQC, 4, 128, 1024], BF16,
                                     isOutput=False)
    yt_d = nc.declare_dram_parameter("yt", [NQC, 128, 2048], BF16,
                                     isOutput=True)


    with tile.TileContext(nc) as tc:
        with tc.tile_pool(name="sb", bufs=1) as sb, \
             tc.tile_pool(name="ps", bufs=2, space="PSUM") as ps:

            # ---------------- DMA: biases + weights first ----------------
            bq_sb, bk_sb = [], []
            for m in range(2):
                t = sb.tile([128, 1], F32, tag=f"bq{m}", name=f"bq{m}")
                nc.sync.dma_start(out=t, in_=bq_d[m])
                bq_sb.append(t)
                t = sb.tile([128, 1], F32, tag=f"bk{m}", name=f"bk{m}")
                nc.sync.dma_start(out=t, in_=bk_d[m])
                bk_sb.append(t)
            bvb_sb = sb.tile([128, 256], BF16, tag="bvb")
            nc.sync.dma_start(out=bvb_sb, in_=bvb_d[:, :])

            # DMA issue order = arrival order = the order compute needs it:
            # wk -> xk (kproj) -> wv -> xv (vproj) -> masks 0-2 -> wq ->
            # xq-n0 (qproj/scores) -> masks 3-5 -> wo.
            wk_sb = sb.tile([128, 2048], BF16, tag="wk")
            nc.sync.dma_start(out=wk_sb, in_=wk_d[:, :])
            xk_sb = []
            for c in range(CT):
                t = sb.tile([128, S], BF16, tag=f"xk{c}", name=f"xk{c}")
                nc.sync.dma_start(out=t, in_=xk_d[c])
                xk_sb.append(t)
            wv_sb = sb.tile([128, 2048], BF16, tag="wv")
            nc.sync.dma_start(out=wv_sb, in_=wv_d[:, :])
            xv_sb = []
            for c in range(CT):
                t = sb.tile([128, S], BF16, tag=f"xv{c}", name=f"xv{c}")
                nc.sync.dma_start(out=t, in_=xv_d[c])
                xv_sb.append(t)

            # masks: tile index M = qc*4 + g covers kt 4g..4g+3 of chunk qc.
            mk_sb = {}

            def mask_dma(M):
                qc, g = M // 4, M % 4
                t = sb.tile([128, 1024], BF16, tag="mask", name="mask", bufs=6)
                nc.sync.dma_start(out=t, in_=mk_d[qc, g])
                mk_sb[M] = t

            for M in range(3):
                mask_dma(M)

            wq_sb = sb.tile([128, 2048], BF16, tag="wq")
            nc.sync.dma_start(out=wq_sb, in_=wq_d[:, :])
            xq_sb = []
            for c in range(CT):
                t = sb.tile([128, S], BF16, tag=f"xq{c}", name=f"xq{c}")
                nc.sync.dma_start(out=t[:, 0:512], in_=xq_d[c][:, 0:512])
                xq_sb.append(t)
            for M in range(3, 6):
                mask_dma(M)
            wo_sb = sb.tile([128, 2048], BF16, tag="wo")
            nc.sync.dma_start(out=wo_sb, in_=wo_d[:, :])

            # ---------------- persistent SBUF intermediates ----------------
            khT = [[sb.tile([128, 512], BF16, tag=f"khT{m}_{n}",
                            name=f"khT{m}_{n}") for n in range(4)]
                   for m in range(2)]
            qhT = [[sb.tile([128, 512], BF16, tag=f"qhT{m}_{n}",
                            name=f"qhT{m}_{n}") for n in range(4)]
                   for m in range(2)]
            vha = [sb.tile([128, VHA_W], BF16, tag=f"vha{kt}",
                           name=f"vha{kt}") for kt in range(KT)]
            OT = [sb.tile([128, S], BF16, tag=f"OT{p}", name=f"OT{p}")
                  for p in range(2)]

            # ---------------- PE warmup + exp table prewarm (FIRST: the
            # warm memset gates the warmup matmuls, so it must precede the
            # 64 vha memsets on the gpsimd queue) ----------------
            warm = sb.tile([128, 512], BF16, tag="warm")
            nc.gpsimd.memset(warm[:, :], 0.0)
            # selector weights for the norm broadcast matmuls:
            # sel[0, 0:64]=1 (A rows), sel[0, 192:256]=1 (B rows)
            sel = sb.tile([1, 256], BF16, tag="sel")
            nc.gpsimd.memset(sel[0:1, 0:64], 1.0)
            nc.gpsimd.memset(sel[0:1, 64:192], 0.0)
            nc.gpsimd.memset(sel[0:1, 192:256], 1.0)
            wps = ps.tile([128, 512], F32, tag="ax", name="wps")
            for i in range(12):
                nc.tensor.matmul(wps, warm[:, 0:128], warm[:, :],
                                 start=True, stop=True)
            prew = sb.tile([128, 32], BF16, tag="prew")
            nc.scalar.activation(out=prew, in_=warm[:, 0:32],
                                 func=mybir.ActivationFunctionType.Exp,
                                 scale=1.0)

            for kt in range(KT):
                for p in range(2):
                    base = p * 193
                    nc.gpsimd.memset(vha[kt][:, base + 64:base + 65], 1.0)
                    nc.gpsimd.memset(vha[kt][:, base + 97:base + 98], 1.0)
                    nc.gpsimd.memset(vha[kt][:, base + 65:base + 97], 0.0)
                    nc.gpsimd.memset(vha[kt][:, base + 98:base + 129], 0.0)

            # ---------------- k projection: 8 groups, c-outer ----------------
            # groups (m, n): n0 -> sc-tile halves, n1 -> sc, n2 -> po, n3 -> ax
            psk = [ps.tile([128, 1024], F32, tag="sc", name=f"psk{n}")
                   for n in range(2)]
            pso = [ps.tile([128, 512], F32, tag="po", name=f"pso{m}")
                   for m in range(2)]
            psa = [ps.tile([128, 512], F32, tag="ax", name=f"psa{m}")
                   for m in range(2)]

            def kgroup_ap(m, n):
                if n < 2:
                    return psk[n][:, m * 512:(m + 1) * 512]
                return (pso if n == 2 else psa)[m][:, :]

            for c in range(CT):
                for n in range(4):
                    for m in range(2):
                        nc.tensor.matmul(
                            kgroup_ap(m, n),
                            wk_sb[:, c * 256 + m * 128:c * 256 + (m + 1) * 128],
                            xk_sb[c][:, n * 512:(n + 1) * 512],
                            start=(c == 0), stop=(c == CT - 1))
            for n in range(4):
                for m in range(2):
                    bb = bk_sb[m][:, 0:1]
                    nc.vector.tensor_tensor(
                        out=khT[m][n], in0=kgroup_ap(m, n),
                        in1=_bc(bb, [list(bb.ap[0]), [0, 512]]),
                        op=mybir.AluOpType.add)

            # ---------------- v projection: c-outer waves over kt ----------
            # 8 kt-pair psum groups spread over all three tags (ramp only):
            # j0..3 -> two sc tiles (halves), j4,j5 -> po, j6,j7 -> ax.
            vsc = [ps.tile([128, 1024], F32, tag="sc", name=f"vsc{i}")
                   for i in range(2)]
            vpo = [ps.tile([128, 512], F32, tag="po", name=f"vpo{i}")
                   for i in range(2)]
            vax = [ps.tile([128, 512], F32, tag="ax", name=f"vax{i}")
                   for i in range(2)]

            def vps(j):
                if j < 4:
                    return vsc[j // 2][:, (j % 2) * 512:(j % 2 + 1) * 512]
                if j < 6:
                    return vpo[j - 4][:, :]
                return vax[j - 6][:, :]

            for c in range(CT):
                for j in range(KT // 2):
                    for kk in range(2):
                        # one start/stop per BANK: start only on (c0, kk0),
                        # stop only on (c7, kk1); flags=0 overwrites regions
                        # whose has_written bit is clear, so kk1@c0 is safe.
                        kt = 2 * j + kk
                        nc.tensor.matmul(
                            vps(j)[:, kk * 256:(kk + 1) * 256],
                            xv_sb[c][:, kt * 128:(kt + 1) * 128],
                            wv_sb[:, c * 256:(c + 1) * 256],
                            start=(c == 0 and kk == 0),
                            stop=(c == CT - 1 and kk == 1))
            def vha_cast(j, kk):
                    kt = 2 * j + kk
                    # A blocks (heads 0,2) then B blocks (heads 1,3), 3D APs
                    src = vps(j)[:, kk * 256:kk * 256 + 64]
                    dstA = vha[kt][:, 0:64]
                    nc.vector.tensor_tensor(
                        out=_bc(dstA, [list(dstA.ap[0]), [193, 2], [1, 64]]),
                        in0=_bc(src, [list(src.ap[0]), [128, 2], [1, 64]]),
                        in1=_bc(bvb_sb[:, 0:64],
                                [list(bvb_sb.ap[0]), [128, 2], [1, 64]]),
                        op=mybir.AluOpType.add)
                    srcB = vps(j)[:, kk * 256 + 64:kk * 256 + 128]
                    dstB = vha[kt][:, 129:193]
                    nc.vector.tensor_tensor(
                        out=_bc(dstB, [list(dstB.ap[0]), [193, 2], [1, 64]]),
                        in0=_bc(srcB, [list(srcB.ap[0]), [128, 2], [1, 64]]),
                        in1=_bc(bvb_sb[:, 64:128],
                                [list(bvb_sb.ap[0]), [128, 2], [1, 64]]),
                        op=mybir.AluOpType.add)
            # NOTE: vha_cast emission deferred until after the qproj-n0
            # casts so the 32 vha casts don't delay qhT on the DVE FIFO.

            # ---------------- q projection chunk n ----------------
            qpq = {}

            def qproj_step(n, c):
                # one c-wave (2 MMs); c==0 allocates, c==CT-1 adds the casts
                if c == 0:
                    qpq[n] = [ps.tile([128, 512], F32, tag="ax",
                                      name=f"pq{n}_{m}") for m in range(2)]
                for m in range(2):
                    nc.tensor.matmul(
                        qpq[n][m],
                        wq_sb[:, c * 256 + m * 128:c * 256 + (m + 1) * 128],
                        xq_sb[c][:, n * 512:(n + 1) * 512],
                        start=(c == 0), stop=(c == CT - 1))
                if c == CT - 1:
                    for m in range(2):
                        bb = bq_sb[m][:, 0:1]
                        nc.vector.tensor_tensor(
                            out=qhT[m][n], in0=qpq[n][m],
                            in1=_bc(bb, [list(bb.ap[0]), [0, 512]]),
                            op=mybir.AluOpType.add)

            for c in range(CT):
                qproj_step(0, c)
            # vha casts AFTER the qproj casts (DVE FIFO order): scores only
            # need khT/qhT; vha is first needed by PV at slot 6 of chunk 0.
            for j in range(KT // 2):
                for kk in range(2):
                    vha_cast(j, kk)

            # ---------------- attention + trailing work ----------------
            # sc col layout per (qc, kt): [h0 | h2 | h1 | h3] x 256 q.
            A_COL = {0: 0, 1: 256}        # pair -> A-head col base
            B_COL = {0: 512, 1: 768}      # pair -> B-head col base

            po = {}        # (qc, p) -> psum tile
            pt2 = {}       # (qc, ktpair) -> sbuf tile
            rbc = {}       # (qc, p) -> sbuf [128, 256] f32
            state = {}

            def emit_scores(qc, kt):
                sc = ps.tile([128, 1024], F32, tag="sc", name="sc")
                n, qoff = qc // 2, (qc % 2) * 256
                kslice = slice((kt % 4) * 128, (kt % 4 + 1) * 128)
                for p in range(2):
                    for ab in range(2):
                        rows = slice(ab * 64, (ab + 1) * 64)
                        col = A_COL[p] if ab == 0 else B_COL[p]
                        nc.tensor.matmul(
                            sc[:, col:col + 256],
                            khT[p][kt // 4][rows, kslice],
                            qhT[p][n][rows, qoff:qoff + 256],
                            start=True, stop=True)
                return sc

            def emit_exp_mask(qc, kt, sc):
                j = kt // 2
                if kt % 2 == 0:
                    pt2[(qc, j)] = sb.tile([128, 2048], BF16, tag="pt2",
                                           name="pt2", bufs=5)
                t = pt2[(qc, j)]
                nc.scalar.activation(
                    out=t[:, (kt % 2) * 1024:(kt % 2 + 1) * 1024], in_=sc,
                    func=mybir.ActivationFunctionType.Exp, scale=float(SCALE))
                if kt % 2 == 1:
                    mkt = mk_sb[qc * 4 + kt // 4]
                    moff = ((kt - 1) % 4) * 256
                    msrc = mkt[:, moff:moff + 256]
                    nc.vector.tensor_tensor(
                        out=t, in0=t,
                        in1=_bc(msrc, [list(msrc.ap[0]), [256, 2], [0, 4],
                                       [1, 256]]),
                        op=mybir.AluOpType.mult)

            def emit_pv(qc, kt):
                t = pt2[(qc, kt // 2)]
                base = (kt % 2) * 1024
                for p in range(2):
                    # A and B share one bank: single start (A@kt0) / stop
                    # (B@kt15); B@kt0 overwrites via clear has_written bits.
                    # A uses M=128 (into the B-block zeros) so the start
                    # marks every partition of the bank; partitions 65:128
                    # of cols 0:256 accumulate unused garbage.
                    vb = p * 193
                    nc.tensor.matmul(
                        po[(qc, p)][:, 0:256],
                        vha[kt][:, vb:vb + 128],
                        t[:, base + A_COL[p]:base + A_COL[p] + 256],
                        start=(kt == 0), stop=False)
                    nc.tensor.matmul(
                        po[(qc, p)][:, 256:512],
                        vha[kt][:, vb + 65:vb + 193],
                        t[:, base + B_COL[p]:base + B_COL[p] + 256],
                        start=False, stop=(kt == KT - 1))

            def emit_sums(qc):
                t = sb.tile([1, 1024], F32, tag="ss", name="ss", bufs=2)
                for p in range(2):
                    nc.vector.tensor_copy(
                        out=t[0:1, p * 512:p * 512 + 256],
                        in_=po[(qc, p)][64:65, 0:256])
                    nc.vector.tensor_copy(
                        out=t[0:1, p * 512 + 256:(p + 1) * 512],
                        in_=po[(qc, p)][32:33, 256:512])
                state[("ss", qc)] = t
                # norm-chain DMAs on the sync HWDGE queue (SWDGE measured
                # ~5us extra latency per hop); emitted early (kt 0/2) so
                # their queue-head waits are short.
                rg = sb.tile([128, 8], F32, tag="rg", name="rg", bufs=2)
                nc.sync.dma_start(out=rg, in_=t[0:1, :])
                state[("rg", qc)] = rg

            def emit_recip(qc):
                rr = sb.tile([128, 8], BF16, tag="rr", name="rr", bufs=2)
                with nc.allow_low_precision(reason="1/rowsum in bf16 is "
                                            "well within the 2e-2 budget"):
                    nc.vector.reciprocal(out=rr, in_=state[("rg", qc)])
                # scatter back to one SBUF row, then broadcast across
                # partitions with K=1 matmuls against the constant selector
                # (A values to rows 0:64, B to 64:128; the zero-padded
                # selector halves merge via per-element has_written).
                rrow = sb.tile([1, 1024], BF16, tag="rrow", name="rrow", bufs=2)
                nc.sync.dma_start(out=rrow, in_=rr)
                rbp = ps.tile([128, 512], F32, tag="ax", name="rbp")
                for p in range(2):
                    nc.tensor.matmul(
                        rbp[:, p * 256:(p + 1) * 256],
                        sel[0:1, 0:128],
                        rrow[0:1, p * 512:p * 512 + 256],
                        start=(p == 0), stop=False)
                    nc.tensor.matmul(
                        rbp[:, p * 256:(p + 1) * 256],
                        sel[0:1, 128:256],
                        rrow[0:1, p * 512 + 256:(p + 1) * 512],
                        start=False, stop=(p == 1))
                t = sb.tile([128, 512], F32, tag="rbc", name="rbc", bufs=2)
                nc.vector.tensor_copy(out=t, in_=rbp)
                rbc[qc] = t

            def emit_ot(qc, p):
                qsl = slice(qc * 256, (qc + 1) * 256)
                csl = slice(p * 256, (p + 1) * 256)
                nc.vector.tensor_tensor(
                    out=OT[p][0:64, qsl],
                    in0=po[(qc, p)][0:64, 0:256],
                    in1=rbc[qc][0:64, csl],
                    op=mybir.AluOpType.mult)
                nc.vector.tensor_tensor(
                    out=OT[p][64:128, qsl],
                    in0=po[(qc, p)][64:128, 256:512],
                    in1=rbc[qc][64:128, csl],
                    op=mybir.AluOpType.mult)

            yts_cur = {}

            def emit_outproj(qc, otp):
                ax = ps.tile([128, 512], F32, tag="ax", name="axo")
                for half in range(2):
                    ot = 2 * otp + half
                    for p in range(2):
                        nc.tensor.matmul(
                            ax[:, half * 256:(half + 1) * 256],
                            wo_sb[:, p * 1024 + ot * 128:
                                  p * 1024 + (ot + 1) * 128],
                            OT[p][:, qc * 256:(qc + 1) * 256],
                            start=(p == 0), stop=(p == 1))
                if otp == 0:
                    yts_cur[qc] = sb.tile([128, 2048], BF16, tag="yts",
                                          name="yts", bufs=2)
                yts = yts_cur[qc]
                nc.vector.tensor_copy(
                    out=yts[:, otp * 512:(otp + 1) * 512], in_=ax)
                if otp == 3:
                    nc.sync.dma_start(out=yt_d[qc], in_=yts)

            # main loop: per chunk qc, slots kt=0..15 pace the emission.
            for qc in range(NQC):
                for p in range(2):
                    po[(qc, p)] = ps.tile([128, 512], F32, tag="po",
                                          name=f"po{p}")
                for kt in range(KT):
                    u = qc * KT + kt  # global slot
                    # JIT DMAs (at u%4==3 so they queue behind, not ahead
                    # of, the latency-critical norm-chain DMAs at kt 0/2)
                    if u % 4 == 3:
                        M = u // 4 + 6
                        if M < NQC * 4:
                            mask_dma(M)
                    if u == 3:
                        for c in range(CT):
                            nc.sync.dma_start(out=xq_sb[c][:, 512:2048],
                                              in_=xq_d[c][:, 512:2048])
                    sc = emit_scores(qc, kt)
                    emit_exp_mask(qc, kt, sc)
                    # deferred norm/outproj for previous chunk; emitted as
                    # early as its dependencies allow so the po banks free
                    # before PV(qc, 0) at slot 6.
                    if qc > 0:
                        pq = qc - 1
                        if kt == 0:
                            emit_sums(pq)
                        elif kt == 2:
                            emit_recip(pq)
                        elif kt == 4:
                            emit_ot(pq, 0)
                        elif kt == 5:
                            emit_ot(pq, 1)
                        elif kt in (7, 9, 11, 13):
                            emit_outproj(pq, (kt - 7) // 2)
                    if qc in (1, 3, 5) and kt < CT:
                        qproj_step((qc + 1) // 2, kt)
                    # PV with lag 6 (waits po release by norm of qc-1)
                    if kt >= 6:
                        emit_pv(qc, kt - 6)
                for kt in range(KT - 6, KT):
                    emit_pv(qc, kt)
            emit_sums(NQC - 1)
            emit_recip(NQC - 1)
            emit_ot(NQC - 1, 0)
            emit_ot(NQC - 1, 1)
            for otp in range(4):
                emit_outproj(NQC - 1, otp)

    nc.compile()
    return nc


_NC_CACHE = None


def get_nc():
    global _NC_CACHE
    if _NC_CACHE is None:
        _NC_CACHE = build_nc()
    return _NC_CACHE


def prep_in_maps(q, k, v, mask, Wq, bq, Wk, bk, Wv, bv, Wo, bo):
    q = np.asarray(q, np.float32)
    k = np.asarray(k, np.float32)
    v = np.asarray(v, np.float32)
    mask = np.asarray(mask)
    WqT = np.asarray(Wq, np.float32).T
    WkT = np.asarray(Wk, np.float32).T
    WvT = np.asarray(Wv, np.float32).T
    WoT = np.asarray(Wo, np.float32).T
    bq = np.asarray(bq, np.float32)
    bk = np.asarray(bk, np.float32)
    bv = np.asarray(bv, np.float32)

    xT = {}
    mkw = {}
    for b in range(B):
        xT[b] = tuple(
            np.ascontiguousarray(a.T).astype(NP_BF16).reshape(CT, 128, S)
            for a in (k[b], v[b], q[b]))
        keep = np.ascontiguousarray(
            (~mask[b, 0]).T.astype(np.float32)).astype(NP_BF16)  # [kpos, q]
        # [g, j, p, qc, q256] -> [qc, g, p, j*256]
        a = keep.reshape(4, 4, 128, NQC, 256)
        mkw[b] = np.ascontiguousarray(
            a.transpose(3, 0, 2, 1, 4).reshape(NQC, 4, 128, 1024))

    def wpack(WT, dsl):
        # [1024, 256] -> [128, 2048] with cols c*256+j
        return np.ascontiguousarray(
            WT[:, dsl].reshape(CT, 128, 256).transpose(1, 0, 2)
            .reshape(128, 2048)).astype(NP_BF16)

    in_maps = []
    for c in range(N_CORES):
        b = c // 4
        ho = c % 4
        dsl = slice(ho * 256, ho * 256 + 256)
        xk, xv, xq = xT[b]
        in_maps.append({
            "xk": xk, "xv": xv, "xq": xq,
            "wq": wpack(WqT, dsl),
            "wk": wpack(WkT, dsl),
            "wv": wpack(WvT, dsl),
            "wo": np.ascontiguousarray(
                WoT[dsl, :].reshape(2, 128, 1024).transpose(1, 0, 2)
                .reshape(128, 2048)).astype(NP_BF16),
            "bq2": np.ascontiguousarray(bq[dsl]).reshape(2, 128, 1)
                .astype(np.float32),
            "bk2": np.ascontiguousarray(bk[dsl]).reshape(2, 128, 1)
                .astype(np.float32),
            "bvb": np.ascontiguousarray(
                np.broadcast_to(bv[dsl], (128, 256))).astype(NP_BF16),
            "mk": mkw[b],
        })
    return in_maps


def assemble_yT(yt):
    # yt [NQC, 128, 2048] -> yT [1024, 2048]; cols = otp*512 + half*256 + q,
    # y-dim = otp*256 + half*128 + part
    a = np.asarray(yt, np.float32).reshape(NQC, 128, 4, 2, 256)
    return a.transpose(2, 3, 1, 0, 4).reshape(DIM, S)


def gather_output(results, bo):
    bo = np.asarray(bo, np.float32)
    y = np.zeros((B, S, DIM), np.float32)
    for c in range(N_CORES):
        y[c // 4] += assemble_yT(results[c]["yt"]).T
    y += bo[None, None, :]
    return y


def kernel(**inputs):
    nc = get_nc()
    in_maps = prep_in_maps(**{k_: inputs[k_] for k_ in (
        "q", "k", "v", "mask", "Wq", "bq", "Wk", "bk", "Wv", "bv", "Wo", "bo")})
    res = bass_utils.run_bass_kernel_spmd(nc, in_maps, list(range(N_CORES)))
    return gather_output(res.results, inputs["bo"])
